# revision 9
# baseline (speedup 1.0000x reference)
# Trainium2 Bass kernel for nn_CapsuleLayer_62706522521966.
#
# Math: the reference's routing loop is dead code — softmax over a singleton
# axis (b_log is [I, O, 1], softmax on axis=2) yields all-ones coupling
# coefficients on every iteration, so the output is exactly
#     out = squash(einsum('bic,iocu->bou', x, w))[:, :, None, :]
# i.e. a single [B, I*C] @ [I*C, O*U] matmul followed by a tiny squash.
#
# Sharding: the O=32 output-capsule dim is split across the 8 NeuronCores
# (4 capsules each). Each core reads its own slice of w plus a replicated
# x^T — no collectives; the host concatenates the 8 slices.
#
# Perf notes:
#  - Matmul operands are cast to fp16 on the host (PSUM still accumulates
#    fp32): fp32 PE matmul is emulated as 2 HW matmuls (hi/lo) and fp32
#    doubles DMA bytes. fp16 keeps max rel err ~4e-4.
#  - Both operands are pre-permuted host-side into partition-major layouts
#    so every DMA reads contiguous HBM per partition.
#  - M=32 only fills a quarter of the PE array, so k-chunks are packed
#    4-at-a-time into the four 32-column groups (tile_position col-tiling),
#    accumulating into four partition slices of one PSUM bank; a final
#    [128->32] fold matmul with a stacked-identity lhsT sums the slices.
#  - w DMAs alternate between the SP and ACT HWDGE rings (issue is FIFO per
#    ring); x goes through SWDGE (gpsimd). First two w tiles are halved so
#    the PE starts after ~0.6 MB instead of ~1.5 MB.

from contextlib import ExitStack

import numpy as np

import concourse.bass as bass  # noqa: F401  (registers AP machinery)
import concourse.tile as tile
from concourse import bacc, mybir
from concourse.bass_utils import run_bass_kernel_spmd

B, I, O, C, U = 32, 2048, 32, 16, 32
N_CORES = 8
O_PER = O // N_CORES            # 4 output capsules per core
N = O_PER * U                   # 128 free (n) elements per core
K = I * C                       # 32768 contraction length
P = 128                         # SBUF partitions per k-chunk
KC = K // P                     # 256 k-chunks
XG = 64                         # k-chunks per x DMA tile (512 KB fp16)
# w DMA tiles as (first_chunk, n_chunks): two half tiles to ramp up, then 1MB
W_TILES = [(0, 16), (16, 16)] + [(32 + 32 * k, 32) for k in range(7)]
F32 = mybir.dt.float32
F16 = mybir.dt.float16
NP_IN = np.float16

_NC_CACHE: dict = {}


def _build_nc():
    nc = bacc.Bacc("TRN2", target_bir_lowering=False, debug=False)

    xt = nc.dram_tensor("xt", [P, KC * B], F16, kind="ExternalInput")
    wt = nc.dram_tensor("wt", [P, KC * N], F16, kind="ExternalInput")
    id4 = nc.dram_tensor("id4", [P, B], F32, kind="ExternalInput")
    out_d = nc.dram_tensor("out", [B, N], F32, kind="ExternalOutput")

    with tile.TileContext(nc) as tc:
        with ExitStack() as ctx:
            xpool = ctx.enter_context(tc.tile_pool(name="xpool", bufs=4))
            wpool = ctx.enter_context(tc.tile_pool(name="wpool", bufs=6))
            cpool = ctx.enter_context(tc.tile_pool(name="cpool", bufs=1))
            pspool = ctx.enter_context(
                tc.tile_pool(name="pspool", bufs=1, space="PSUM")
            )
            spool = ctx.enter_context(tc.tile_pool(name="spool", bufs=1))

            # Preload the Sqrt ACT table while PE/DMA do the real work, so
            # the epilogue doesn't pay the ~1.3us table load.
            warm = spool.tile([1, 1], F32)
            nc.vector.memset(warm, 1.0)
            warm2 = spool.tile([1, 1], F32)
            nc.scalar.sqrt(warm2, warm)

            # stacked identity for the final [128->32] partition fold
            id_sb = cpool.tile([P, B], F32)
            nc.gpsimd.dma_start(out=id_sb, in_=id4[:, :])

            # four 32-partition accumulator slices in one PSUM bank
            pc = pspool.tile([P, N], F32)
            x_tiles = []
            for c0, cnt in W_TILES:
                if c0 % XG == 0:
                    xi = c0 // XG
                    x_t = xpool.tile([P, XG, B], F16)
                    nc.gpsimd.dma_start(
                        out=x_t,
                        in_=xt[:, xi * XG * B : (xi + 1) * XG * B].rearrange(
                            "p (c b) -> p c b", b=B
                        ),
                    )
                    x_tiles.append(x_t)
                w_full = wpool.tile([P, 32 * N], F16, tag="w_t", name="w_t")
                w_t = w_full[:, : cnt * N]
                eng = nc.sync if (c0 // 32) % 2 == 0 else nc.scalar
                eng.dma_start(
                    out=w_t, in_=wt[:, c0 * N : (c0 + cnt) * N]
                )
                for g in range(cnt):
                    c = c0 + g
                    j = c % 4
                    nc.tensor.matmul(
                        pc[32 * j : 32 * (j + 1), :],
                        lhsT=x_tiles[c // XG][:, c % XG, :],
                        rhs=w_t[:, g * N : (g + 1) * N],
                        start=(c < 4),
                        stop=(c >= KC - 4),
                        tile_position=(0, 32 * j),
                    )

            # fold the 4 partition slices: s = ID4^T @ pc_sb  (exact: weights
            # are 0/1 so the fp32-emulated matmul loses nothing)
            pc_sb = spool.tile([P, N], F32)
            nc.vector.tensor_copy(pc_sb, pc)
            ps = pspool.tile([B, N], F32)
            nc.tensor.matmul(ps, lhsT=id_sb, rhs=pc_sb, start=True, stop=True)

            # squash: v = s * n / (1 + n^2), n = ||s|| over the unit dim.
            s_sb = spool.tile([B, N], F32)
            nc.vector.tensor_copy(s_sb, ps)
            sq = spool.tile([B, N], F32)
            nc.vector.tensor_mul(sq, s_sb, s_sb)
            ssq = spool.tile([B, O_PER], F32)
            nc.vector.reduce_sum(
                ssq,
                sq.rearrange("b (o u) -> b o u", u=U),
                axis=mybir.AxisListType.X,
            )
            nrm = spool.tile([B, O_PER], F32)
            nc.scalar.sqrt(nrm, ssq)
            den = spool.tile([B, O_PER], F32)
            nc.vector.tensor_scalar_add(den, ssq, 1.0)
            rden = spool.tile([B, O_PER], F32)
            nc.vector.reciprocal(rden, den)
            fac = spool.tile([B, O_PER], F32)
            nc.vector.tensor_mul(fac, nrm, rden)
            v = spool.tile([B, N], F32)
            for o in range(O_PER):
                nc.vector.tensor_scalar_mul(
                    v[:, o * U : (o + 1) * U],
                    s_sb[:, o * U : (o + 1) * U],
                    fac[:, o : o + 1],
                )
            nc.sync.dma_start(out=out_d[:, :], in_=v)

    nc.compile()
    return nc


def _get_nc():
    if "nc" not in _NC_CACHE:
        _NC_CACHE["nc"] = _build_nc()
    return _NC_CACHE["nc"]


def _prep_inputs(x: np.ndarray, w: np.ndarray):
    x = np.ascontiguousarray(x, dtype=np.float32)
    w = np.ascontiguousarray(w, dtype=np.float32)
    # x^T in partition-major layout: xt[p, ck, b] = x_flat[b, ck*128 + p]
    x_flat = x.reshape(B, K)
    xt_host = np.ascontiguousarray(
        x_flat.T.reshape(KC, P, B).transpose(1, 0, 2), dtype=NP_IN
    ).reshape(P, KC * B)
    id4_host = np.tile(np.eye(B, dtype=np.float32), (P // B, 1))
    in_maps = []
    for j in range(N_CORES):
        wsh = w[:, j * O_PER : (j + 1) * O_PER]  # [I, O_PER, C, U]
        # wt[p=(i_sub,c), ck, n=(o,u)] = w[ck*8+i_sub, o, c, u]
        wt_host = np.ascontiguousarray(
            wsh.reshape(KC, P // C, O_PER, C, U).transpose(1, 3, 0, 2, 4),
            dtype=NP_IN,
        ).reshape(P, KC * N)
        in_maps.append({"xt": xt_host, "wt": wt_host, "id4": id4_host})
    return in_maps


def run(inputs: dict, **spmd_kwargs):
    """Build+run the SPMD kernel; returns (full_output, BassKernelResults)."""
    nc = _get_nc()
    in_maps = _prep_inputs(inputs["x"], inputs["w"])
    res = run_bass_kernel_spmd(nc, in_maps, list(range(N_CORES)), **spmd_kwargs)
    parts = [res.results[j]["out"].reshape(B, O_PER, U) for j in range(N_CORES)]
    v = np.concatenate(parts, axis=1)  # [B, O, U]
    return np.ascontiguousarray(v[:, :, None, :]).astype(np.float32), res


def kernel(x: np.ndarray, w: np.ndarray) -> np.ndarray:
    out, _ = run({"x": x, "w": w})
    return out
